# revision 32
# baseline (speedup 1.0000x reference)
"""AttentionGCNConv edge kernel for 8 Trainium2 NeuronCores.

Strategy (edge-sharded SPMD, no cross-core reduction):
  * Nodes sharded 8-ways; edges bucketed by owner core (col // ns) and
    sorted by col. Each node's edge list is padded to a multiple of 4 so
    every gathered table row serves a QUAD of 4 edge slots.
  * Node phase per core: h = x@W_lin+b, gn = exp(f(h)) via a host-fitted
    factored polynomial (exact 16-relu fallback), then a 256-byte bf16
    table row per node: [nodeprod16 = repeat2(h[:8])*gn, h_hi8, G=sum gn].
  * Edge phase per tile: K-packed matmuls (8 edge-blocks against a
    block-diagonal W_edge, K=72) -> ea; exp(f(ea)) via the polynomial;
    row-sum -> easum.
  * Gather: dma_gather (mlp-library CounterMachine path) fetches one
    256B row per quad, 1024 idxs/call, round-robin over 4 SWDGE queues
    (parallel Q7 pairs).
  * Combine: D = G + easum, R = 1/D; out left = nodeprod * R;
    out right = repeat2(h_hi * R) * gea. Output written bf16, host
    casts to f32 and unpermutes.
"""
import numpy as np


# ---------------------------------------------------------------------------
# problem constants (hardcoded per the task statement)
# ---------------------------------------------------------------------------
N_NODES = 100000
E_EDGES = 1000000
IN_C = 64
C = 16          # OUT_C
ED = 8          # EDGE_D
NCORES = 8
P = 128
KPACK = 8       # edge-blocks stacked along matmul contraction dim
QUAD = 4        # edge slots per gathered table row
TW = 128        # table row width in bf16 (256B)
NI_CALL = 1024  # idxs per dma_gather call (HW limit ~1024)
NQUEUES = 4


class Cfg:
    def __init__(self, n_nodes, e_edges, ncores, kt, in_c=IN_C, ecap=None):
        self.ncores = ncores
        self.in_c = in_c
        self.kt = kt                     # edge slots per partition per tile
        assert kt % KPACK == 0 and kt % QUAD == 0
        assert (P * kt) % (QUAD * NI_CALL) == 0
        self.calls_per_tile = (P * kt) // (QUAD * NI_CALL)
        tile_slots = P * kt
        cap = ecap if ecap is not None else (e_edges // ncores + e_edges // 8)
        self.ntiles = -(-cap // tile_slots)
        self.pk = self.ntiles * kt       # slots per partition
        self.pk8 = self.pk // KPACK
        self.nqp = self.pk // QUAD       # quads per partition
        self.s_dev = P * self.pk
        self.ncalls = self.ntiles * self.calls_per_tile
        self.icols = self.ncalls * (NI_CALL // 16)
        # node shard: multiple of 128
        ns = -(-n_nodes // ncores)
        self.ns = -(-ns // P) * P
        self.nchunks = self.ns // P
        self.n_nodes = n_nodes
        self.e_edges = e_edges


# ---------------------------------------------------------------------------
# host-side derived parameters
# ---------------------------------------------------------------------------
def _f_scalar(s, w1, b1, w2, b2):
    z = s[..., None] * w1 + b1
    return (np.maximum(z, 0.0) * w2).sum(-1) + b2[0]


def fit_poly_factors(w1, b1, w2, b2, lo, hi, tol=3.0e-3, tol_max=6e-3):
    """Fit exp(f(s)) on [lo, hi] by a polynomial that factors into real
    quadratics (a*s+b)^2 + v.  Returns (factors, max_rel_err) or None."""
    grid = np.linspace(lo, hi, 8192)
    target = np.exp(_f_scalar(grid, w1, b1, w2, b2))
    best = None
    for deg in (4, 6, 8, 10, 12, 14, 16):
        ch = np.polynomial.chebyshev.Chebyshev.fit(grid, target, deg)
        p = ch.convert(kind=np.polynomial.Polynomial)
        c_lead = p.coef[-1]
        if c_lead <= 0:
            continue
        roots = p.roots()
        creal = sorted([r.real for r in roots if abs(r.imag) < 1e-12])
        ccplx = [r for r in roots if r.imag > 1e-12]
        if len(creal) % 2 != 0:
            continue
        quads = [(-r.real, r.imag ** 2) for r in ccplx]
        for i in range(0, len(creal), 2):
            r1, r2 = creal[i], creal[i + 1]
            quads.append((-(r1 + r2) / 2.0, -(((r1 - r2) / 2.0) ** 2)))
        nf = len(quads)
        alpha = c_lead ** (1.0 / nf)
        sa = float(np.sqrt(alpha))
        facs = [(sa, sa * u, alpha * v) for (u, v) in quads]
        acc = np.ones_like(grid)
        for (a, b, v) in facs:
            acc = acc * ((a * grid + b) ** 2 + v)
        rel = np.abs(acc - target) / np.abs(target)
        err = float(rel.max())
        if err < tol:
            return facs, err
        if err < tol_max and (best is None or err < best[1]):
            best = (facs, err)
    return best


def derive_params(inputs):
    w1 = np.asarray(inputs["w1"], np.float64)
    b1 = np.asarray(inputs["b1"], np.float64)
    w2 = np.asarray(inputs["w2"], np.float64)
    b2 = np.asarray(inputs["b2"], np.float64)
    W_edge = np.asarray(inputs["W_edge"], np.float64)
    b_edge = np.asarray(inputs["b_edge"], np.float64)
    W_lin = np.asarray(inputs["W_lin"], np.float64)
    b_lin = np.asarray(inputs["b_lin"], np.float64)
    x = np.asarray(inputs["x"], np.float64)
    edge_attr = np.asarray(inputs["edge_attr"], np.float64)

    # node-side exact branch params: w2_k relu(w1_k s + b1_k)
    aw = w1 * np.abs(w2)
    cw = b1 * np.abs(w2)
    sg = np.sign(w2)

    # exact value ranges (host matmuls are cheap) + margin for bf16 drift
    ea = edge_attr @ W_edge + b_edge
    lo_e, hi_e = float(ea.min()), float(ea.max())
    m_e = 0.03 * (hi_e - lo_e) + 0.01
    lo_e, hi_e = lo_e - m_e, hi_e + m_e
    h = x @ W_lin + b_lin
    lo_h, hi_h = float(h.min()), float(h.max())
    m_h = 0.03 * (hi_h - lo_h) + 0.02
    lo_h, hi_h = lo_h - m_h, hi_h + m_h

    fit_e = fit_poly_factors(w1, b1, w2, b2, lo_e, hi_e, tol=5e-3)
    fit_h = fit_poly_factors(w1, b1, w2, b2, lo_h, hi_h, tol=4e-3)

    # edge path: exp(quartic) via nested squares + one linear-add.
    # q(s) ~ f(s); depress (kill cubic) at y = s + B; then
    # q = c4 y^4 + c2 y^2 + c1 y + c0 = sigma*X2 + c1 s + v with
    # X1 = (s+B)^2, X2 = (X1 + gamma)^2, gamma = c2/(2 c4), sigma = c4.
    # node path: exp(octic) via 3 nested squares + one linear-add
    oct_h = None
    try:
        from scipy.optimize import least_squares

        grid_h = np.linspace(lo_h, hi_h, 4096)
        fh = _f_scalar(grid_h, w1, b1, w2, b2)

        def _oct(p, s):
            B, g1, g2, sig, tau, v = p
            return sig * ((((s + B) ** 2 + g1) ** 2 + g2) ** 2) + tau * s + v

        best = None
        inits = [np.array([-1.225, -21.72, -22.58, -2.0e-7, 1.63e-2,
                           float(fh.mean())])]
        rng = np.random.default_rng(1)
        for k in range(12):
            inits.append(np.array([
                rng.normal(0, 1.5), rng.normal(0, 8), rng.normal(0, 30),
                rng.normal(0, 1e-3), rng.normal(0, 0.2), float(fh.mean())]))
        for p0 in inits:
            try:
                r = least_squares(lambda p: _oct(p, grid_h) - fh, p0,
                                  method="lm", max_nfev=2000)
            except Exception:
                continue
            e = float(np.abs(r.fun).max())
            if best is None or e < best[0]:
                best = (e, r.x)
            if e < 5e-3:
                break
        if best is not None and best[0] < 6e-3:
            B, g1, g2, sig, tau, v = [float(x) for x in best[1]]
            oct_h = {"B": B, "g1": g1, "g2": g2, "sigma": sig,
                     "lam": tau / sig, "v": v, "err": best[0]}
    except ImportError:
        pass

    quart_e = None
    grid = np.linspace(lo_e, hi_e, 8192)
    fv = _f_scalar(grid, w1, b1, w2, b2)
    p = np.polynomial.chebyshev.Chebyshev.fit(grid, fv, 4).convert(
        kind=np.polynomial.Polynomial)
    if abs(p.coef[4]) > 1e-12:
        B = float(p.coef[3] / (4 * p.coef[4]))
        py = p(np.polynomial.Polynomial([-B, 1.0]))   # coeffs in y = s + B
        c4, c2, c1, c0 = (float(py.coef[4]), float(py.coef[2]),
                          float(py.coef[1]), float(py.coef[0]))
        gamma = c2 / (2 * c4)
        sigma = c4
        lam = c1 / sigma
        v = c0 + c1 * B - c4 * gamma * gamma
        X2 = ((grid + B) ** 2 + gamma) ** 2
        approx = sigma * X2 + sigma * lam * grid + v
        err = float(np.abs(np.exp(approx) - np.exp(fv)).max()
                    / np.exp(fv).min())
        relerr = float(np.abs(np.expm1(approx - fv)).max())
        if relerr < 6.5e-3:
            quart_e = {"B": B, "gamma": gamma, "sigma": sigma,
                       "lam": lam, "v": v, "err": relerr}
    return {
        "aw": aw, "cw": cw, "sg": sg, "b2": float(b2[0]),
        "lo_e": lo_e, "hi_e": hi_e, "lo_h": lo_h, "hi_h": hi_h,
        "poly_e": None if fit_e is None else fit_e[0],
        "poly_h": None if fit_h is None else fit_h[0],
        "quart_e": quart_e,
        "oct_h": oct_h,
    }


# ---------------------------------------------------------------------------
# graph builder (SPMD, one graph for all cores)
# ---------------------------------------------------------------------------
def build_graph(cfg, dp, acalls):
    from concourse import bass, mybir, library_config
    import concourse.tile as tile

    f32 = mybir.dt.float32
    bf16 = mybir.dt.bfloat16
    i16 = mybir.dt.int16
    ALU = mybir.AluOpType
    ACTF = mybir.ActivationFunctionType

    KB = (ED + 1) * KPACK                 # stacked contraction dim (72)
    NB = C * KPACK                        # block-diag output cols (128)
    KT = cfg.kt
    KT8 = KT // KPACK
    NQT = KT // QUAD                      # quads per partition per tile
    NCH = cfg.nchunks
    GBUFS = 4

    nc = bass.Bass(num_swdge_queues=NQUEUES)
    xt = nc.declare_dram_parameter("xt", [cfg.in_c + 1, cfg.ns], bf16, isOutput=False)
    wlin = nc.declare_dram_parameter("wlin", [cfg.in_c + 1, C], bf16, isOutput=False)
    eat = nc.declare_dram_parameter("eat", [KB, cfg.pk8, P], bf16, isOutput=False)
    wedge = nc.declare_dram_parameter("wedge", [KB, NB], bf16, isOutput=False)
    qidx = nc.declare_dram_parameter("qidx", [P, cfg.icols], i16, isOutput=False)
    cvec = nc.declare_dram_parameter("cvec", [P, 64], f32, isOutput=False)
    out_e = nc.declare_dram_parameter("out", [P, cfg.pk * 2 * C], bf16,
                                      isOutput=True)
    # cvec cols: 0..15 cw_k, 16 b2, 17.. edge-poly b_i, 40.. node-poly b_i

    NCH0_ = cfg.nchunks // 2
    tableA = nc.dram_tensor("tableA", [P * NCH0_, TW], bf16)
    tableB = nc.dram_tensor("tableB", [cfg.ns, TW], bf16)

    def poly_chain(pool, cl_ap, out_ap, facs, bias_col, width, cvec_sb,
                   ones_sb):
        """acc = prod_i ((a_i x + b_i)^2 + v_i); last factor writes out_ap.
        sq double-buffered so ACT(i+1) overlaps the DVE read of sq(i); the
        first factor multiplies a ones tile (STT is ~2x faster than TS)."""
        sqs = [pool.tile([P, width], bf16, tag="sq0", name="sq0"),
               pool.tile([P, width], bf16, tag="sq1", name="sq1")]
        qa = pool.tile([P, width], bf16, tag="qa")
        qb = pool.tile([P, width], bf16, tag="qb")
        nf = len(facs)
        for i, (a, b, v) in enumerate(facs):
            sq = sqs[i % 2]
            nc.scalar.activation(
                out=sq[:], in_=cl_ap, func=ACTF.Square,
                bias=cvec_sb[:, bias_col + i:bias_col + i + 1], scale=float(a),
            )
            src = (ones_sb[:, :width] if i == 0
                   else (qa if i % 2 == 1 else qb)[:])
            dst = out_ap if i == nf - 1 else (qb if i % 2 == 1 else qa)[:]
            nc.vector.scalar_tensor_tensor(
                out=dst, in0=sq[:], scalar=float(v), in1=src,
                op0=ALU.add, op1=ALU.mult)

    def relu_chain(pool, in_ap, out_ap, width, cvec_sb):
        """out = exp(b2 + sum_k sg_k relu(aw_k x + cw_k))."""
        tmp = pool.tile([P, width], f32, tag="rl_t")
        aa = pool.tile([P, width], f32, tag="rl_a")
        ab = pool.tile([P, width], f32, tag="rl_b")
        for k in range(C):
            nc.scalar.activation(
                out=tmp[:], in_=in_ap, func=ACTF.Relu,
                bias=cvec_sb[:, k:k + 1], scale=float(dp["aw"][k]))
            if k == 0:
                nc.vector.tensor_scalar(
                    out=aa[:], in0=tmp[:], scalar1=float(dp["sg"][k]),
                    scalar2=None, op0=ALU.mult)
            else:
                src, dst = (aa, ab) if k % 2 == 1 else (ab, aa)
                nc.vector.scalar_tensor_tensor(
                    out=dst[:], in0=tmp[:], scalar=float(dp["sg"][k]),
                    in1=src[:], op0=ALU.mult, op1=ALU.add)
        fin = aa if C % 2 == 1 else ab
        nc.scalar.activation(
            out=out_ap, in_=fin[:], func=ACTF.Exp,
            bias=cvec_sb[:, 16:17], scale=1.0)

    with tile.TileContext(nc) as tc, nc.allow_low_precision(
            reason="bf16 outputs are within the 2e-2 rel-err budget"):
        with tc.tile_pool(name="const", bufs=1) as constp:
            nc.gpsimd.load_library(library_config.mlp)
            wlin_sb = constp.tile([cfg.in_c + 1, C], bf16)
            nc.sync.dma_start(out=wlin_sb[:], in_=wlin[:])
            wedge_sb = constp.tile([KB, NB], bf16)
            nc.sync.dma_start(out=wedge_sb[:], in_=wedge[:])
            cvec_sb = constp.tile([P, 64], f32)
            nc.sync.dma_start(out=cvec_sb[:], in_=cvec[:])
            qidx_sb = constp.tile([P, cfg.icols], i16)
            nc.sync.dma_start(out=qidx_sb[:], in_=qidx[:])
            ones_sb = constp.tile([P, 2048], bf16)
            nc.vector.memset(ones_sb[:], 1.0)

            NCH0 = NCH // 2                   # chunks in the early table
            # ---------------- fused node + edge pipeline ----------------
            with (
                tc.tile_pool(name="node_sb", bufs=1) as np_sb,
                tc.tile_pool(name="node_ps", bufs=2, space="PSUM") as np_ps,
                tc.tile_pool(name="ea_sb", bufs=2) as ea_sb,
                tc.tile_pool(name="poly_sb", bufs=1) as poly_sb,
                tc.tile_pool(name="ea_keep", bufs=1) as ea_keep,
                tc.tile_pool(name="ea_ps", bufs=3, space="PSUM") as ea_ps,
                tc.tile_pool(name="g_sb", bufs=GBUFS) as g_sb,
                tc.tile_pool(name="o_sb", bufs=2) as o_sb,
            ):
                tlB = tableB[:].rearrange("(p j) c -> p j c", j=NCH)

                def node_half(h):
                    j0, j1 = (0, NCH0) if h == 0 else (NCH0, NCH)
                    ncs = j1 - j0
                    fw = ncs * C
                    xt_sb = np_sb.tile([cfg.in_c + 1, ncs * P], bf16,
                                       tag="xt", name="xt_sb")
                    nc.sync.dma_start(out=xt_sb[:],
                                      in_=xt[:, j0 * P:j1 * P])
                    h_wide = np_sb.tile([P, fw], f32, tag="h_wide",
                                        name="h_wide")
                    for g0 in range(0, ncs, 32):
                        g1 = min(g0 + 32, ncs)
                        pst = np_ps.tile([P, 512], f32, tag="np_ps",
                                         name="pst")
                        for j in range(g0, g1):
                            nc.tensor.matmul(
                                out=pst[:, (j - g0) * C:(j - g0 + 1) * C],
                                lhsT=xt_sb[:, j * P:(j + 1) * P],
                                rhs=wlin_sb[:],
                                start=True, stop=True,
                            )
                        nc.scalar.copy(
                            out=h_wide[:, g0 * C:g1 * C],
                            in_=pst[:, :(g1 - g0) * C],
                        )

                    gn_wide = np_sb.tile([P, fw], f32, tag="gn_wide",
                                         name="gn_wide")
                    if dp["oct_h"] is not None:
                        nx = [np_sb.tile([P, fw], f32, tag=f"nx{i}",
                                         name=f"nx{i}") for i in range(3)]
                        src_ap = h_wide[:]
                        for i in range(3):
                            nc.scalar.activation(
                                out=nx[i][:], in_=src_ap, func=ACTF.Square,
                                bias=cvec_sb[:, 40 + i:41 + i], scale=1.0)
                            src_ap = nx[i][:]
                        tn = np_sb.tile([P, fw], f32, tag="tn", name="tn")
                        nc.vector.scalar_tensor_tensor(
                            out=tn[:], in0=h_wide[:],
                            scalar=float(dp["oct_h"]["lam"]),
                            in1=nx[2][:], op0=ALU.mult, op1=ALU.add)
                        nc.scalar.activation(
                            out=gn_wide[:], in_=tn[:], func=ACTF.Exp,
                            bias=cvec_sb[:, 43:44],
                            scale=float(dp["oct_h"]["sigma"]))
                    elif dp["poly_h"] is not None:
                        poly_chain(np_sb, h_wide[:], gn_wide[:],
                                   dp["poly_h"], 40, fw, cvec_sb, ones_sb)
                    else:
                        relu_chain(np_sb, h_wide[:], gn_wide[:], fw, cvec_sb)

                    # 256B row per node; row r = p*nch + j -> contiguous
                    # per-partition table writes
                    nrow = np_sb.tile([P, ncs, TW], bf16, tag="nrow",
                                      name="nrow")
                    nc.vector.tensor_tensor(
                        out=nrow[:, :, 0:C].rearrange(
                            "p j (h two) -> p j h two", two=2),
                        in0=h_wide[:].rearrange("p (j c) -> p j c", c=C)
                        [:, :, 0:C // 2].unsqueeze(-1).to_broadcast(
                            [P, ncs, C // 2, 2]),
                        in1=gn_wide[:].rearrange(
                            "p (j h two) -> p j h two", h=C // 2, two=2),
                        op=ALU.mult,
                    )
                    nc.vector.tensor_copy(
                        out=nrow[:, :, C:C + C // 2],
                        in_=h_wide[:].rearrange("p (j c) -> p j c", c=C)
                        [:, :, C // 2:C],
                    )
                    g_wide = np_sb.tile([P, ncs], f32, tag="g_wide",
                                        name="g_wide")
                    nc.vector.tensor_reduce(
                        out=g_wide[:],
                        in_=gn_wide[:].rearrange("p (j c) -> p j c", c=C),
                        axis=mybir.AxisListType.X, op=ALU.add,
                    )
                    nc.vector.tensor_copy(
                        out=nrow[:, :, C + C // 2:C + C // 2 + 1],
                        in_=g_wide[:].unsqueeze(-1))

                    if h == 0:
                        tlA = tableA[:].rearrange("(p j) c -> p j c", j=NCH0)
                        nc.scalar.dma_start(out=tlA[:, :, :], in_=nrow[:])
                        nc.scalar.dma_start(out=tlB[:, j0:j1, :],
                                            in_=nrow[:])
                    else:
                        nc.scalar.dma_start(out=tlB[:, j0:j1, :],
                                            in_=nrow[:])

                gea_full = ea_keep.tile([P, cfg.pk * C], bf16)
                easum_full = ea_keep.tile([P, cfg.pk], bf16)

                grows = {}
                qctr = 0
                HW = 1024                     # cols per psum group
                NGRP = (KT8 * NB) // HW       # psum groups per tile
                GK = KT8 // NGRP              # k-packed blocks per group
                LAG = 2

                def phase1(t):
                    nonlocal qctr
                    eat_t = ea_sb.tile([KB, KT8, P], bf16, tag="eat_t",
                                       name="eat_t")
                    nc.sync.dma_start(
                        out=eat_t[:], in_=eat[:, t * KT8:(t + 1) * KT8, :])
                    gea_t = gea_full[:, t * KT * C:(t + 1) * KT * C]
                    for hb in range(NGRP):
                        pse = ea_ps.tile([P, HW], f32, tag="ea_ps", name="pse")
                        for k in range(GK):
                            sb = hb * GK + k
                            nc.tensor.matmul(
                                out=pse[:, k * NB:(k + 1) * NB],
                                lhsT=eat_t[:, sb, :],
                                rhs=wedge_sb[:],
                                start=True, stop=True,
                            )
                        gea_h = gea_t[:, hb * HW:(hb + 1) * HW]
                        if dp["quart_e"] is not None:
                            x1 = poly_sb.tile([P, HW], bf16, tag="sq0",
                                              name="x1")
                            nc.scalar.activation(
                                out=x1[:], in_=pse[:], func=ACTF.Square,
                                bias=cvec_sb[:, 17:18], scale=1.0)
                            x2 = poly_sb.tile([P, HW], bf16, tag="sq1",
                                              name="x2")
                            nc.scalar.activation(
                                out=x2[:], in_=x1[:], func=ACTF.Square,
                                bias=cvec_sb[:, 18:19], scale=1.0)
                            tq = poly_sb.tile([P, HW], f32, tag="qa",
                                              name="tq")
                            nc.vector.scalar_tensor_tensor(
                                out=tq[:], in0=pse[:],
                                scalar=float(dp["quart_e"]["lam"]),
                                in1=x2[:], op0=ALU.mult, op1=ALU.add)
                            nc.scalar.activation(
                                out=gea_h, in_=tq[:], func=ACTF.Exp,
                                bias=cvec_sb[:, 19:20],
                                scale=float(dp["quart_e"]["sigma"]))
                        elif dp["poly_e"] is not None:
                            poly_chain(poly_sb, pse[:], gea_h, dp["poly_e"],
                                       17, HW, cvec_sb, ones_sb)
                        else:
                            relu_chain(poly_sb, pse[:], gea_h, HW, cvec_sb)

                    nc.vector.tensor_reduce(
                        out=easum_full[:, t * KT:(t + 1) * KT],
                        in_=gea_t.rearrange("p (k c) -> p k c", c=C),
                        axis=mybir.AxisListType.X, op=ALU.add,
                    )  # bf16 accumulate: ~0.4% on the softmax denominator

                    grow = g_sb.tile([P, NQT, TW], bf16, tag="grow",
                                     name="grow")
                    for cc in range(cfg.calls_per_tile):
                        call = t * cfg.calls_per_tile + cc
                        col0 = call * (NI_CALL // 16)
                        nc.gpsimd.dma_gather(
                            out_ap=grow[:, cc * (NI_CALL // P):
                                        (cc + 1) * (NI_CALL // P), :],
                            in_ap=(tableA[:] if acalls[call] else tableB[:]),
                            idxs_ap=qidx_sb[:, col0:col0 + NI_CALL // 16],
                            num_idxs=NI_CALL,
                            num_idxs_reg=NI_CALL,
                            elem_size=TW,
                            queue_num=qctr % NQUEUES,
                        )
                        qctr += 1
                    grows[t] = grow

                def combine(t):
                    grow = grows.pop(t)
                    es_t = easum_full[:, t * KT:(t + 1) * KT]
                    gea_t = gea_full[:, t * KT * C:(t + 1) * KT * C]
                    d_t = o_sb.tile([P, KT], f32, tag="d_t")
                    nc.vector.tensor_tensor(
                        out=d_t[:].rearrange("p (q j) -> p q j", j=QUAD),
                        in0=grow[:, :, 24:25].to_broadcast([P, NQT, QUAD]),
                        in1=es_t.rearrange("p (q j) -> p q j", j=QUAD),
                        op=ALU.add,
                    )
                    r_t = o_sb.tile([P, KT], bf16, tag="r_t")
                    nc.vector.reciprocal(out=r_t[:], in_=d_t[:])
                    out_t = o_sb.tile([P, KT, 2 * C], bf16, tag="out_t")
                    # left: nodeprod * R
                    nc.vector.tensor_tensor(
                        out=out_t[:, :, 0:C].rearrange(
                            "p (q j) c -> p q j c", j=QUAD),
                        in0=grow[:, :, 0:C].unsqueeze(2).to_broadcast(
                            [P, NQT, QUAD, C]),
                        in1=r_t[:].rearrange("p (q j) -> p q j", j=QUAD)
                        .unsqueeze(-1).to_broadcast([P, NQT, QUAD, C]),
                        op=ALU.mult,
                    )
                    # wh = h_hi * R  (per slot)
                    wh = o_sb.tile([P, KT, C // 2], bf16, tag="wh")
                    nc.vector.tensor_tensor(
                        out=wh[:].rearrange("p (q j) h -> p q j h", j=QUAD),
                        in0=grow[:, :, C:C + C // 2].unsqueeze(2).to_broadcast(
                            [P, NQT, QUAD, C // 2]),
                        in1=r_t[:].rearrange("p (q j) -> p q j", j=QUAD)
                        .unsqueeze(-1).to_broadcast([P, NQT, QUAD, C // 2]),
                        op=ALU.mult,
                    )
                    # right: repeat2(wh) * gea
                    nc.vector.tensor_tensor(
                        out=out_t[:, :, C:2 * C].rearrange(
                            "p k (h two) -> p k h two", two=2),
                        in0=wh[:].unsqueeze(-1).to_broadcast(
                            [P, KT, C // 2, 2]),
                        in1=gea_t.rearrange("p (k h two) -> p k h two",
                                            h=C // 2, two=2),
                        op=ALU.mult,
                    )
                    nc.sync.dma_start(
                        out=out_e[:].rearrange(
                            "p (k c) -> p k c", c=2 * C)
                        [:, t * KT:(t + 1) * KT, :],
                        in_=out_t[:],
                    )

                node_half(0)
                phase1(0)
                phase1(1)
                node_half(1)
                for t in range(2, cfg.ntiles + LAG):
                    if t < cfg.ntiles:
                        phase1(t)
                    if t - LAG >= 0:
                        combine(t - LAG)
    return nc


# ---------------------------------------------------------------------------
# post-passes
# ---------------------------------------------------------------------------
def _split_multi_waits(nc):
    """This walrus build supports at most one sem-wait per instruction;
    hoist extra waits onto single-wait NoOps inserted just before."""
    from concourse import mybir
    ctr = [0]
    for f in nc.m.functions:
        for bb in f.blocks:
            il = bb.instructions
            new = []
            for inst in il:
                si = inst.sync_info
                waits = list(si.on_wait) if (si is not None and si.on_wait) else []
                if len(waits) > 1:
                    for w in waits[:-1]:
                        ctr[0] += 1
                        nop = mybir.InstNoOp(
                            name=f"splitw-{ctr[0]}", ins=[], outs=[])
                        nop.engine = inst.engine
                        nop.sync_info = mybir.SyncInfo(on_wait=[w], on_update=[])
                        new.append(nop)
                    si.on_wait = [waits[-1]]
                new.append(inst)
            il[:] = new
    return ctr[0]


def _patch_compiler_flags():
    """Enable the vector_dynamic_offsets DGE level; the default flag bundle
    disables it."""
    from concourse.compiler_utils import get_compiler_flags, set_compiler_flags
    flags = list(get_compiler_flags())
    if not flags:
        return
    out = []
    i = 0
    while i < len(flags):
        if flags[i] == "--internal-disable-dge-levels":
            i += 1
            while i < len(flags) and not flags[i].startswith("-"):
                i += 1
            continue
        out.append(flags[i])
        i += 1
    if "--internal-enable-dge-levels" in out:
        j = out.index("--internal-enable-dge-levels")
        if "vector_dynamic_offsets" not in out:
            out.insert(j + 1, "vector_dynamic_offsets")
    set_compiler_flags(out)


# ---------------------------------------------------------------------------
# host prep + entry
# ---------------------------------------------------------------------------
def _tobf16(x):
    import ml_dtypes
    return np.asarray(x, dtype=ml_dtypes.bfloat16)


def quad_layout(col_local_sorted, e_sorted, ns, nqtot):
    """Build quad-packed slot layout for one core.

    Returns (eslot_dev [P, pk] global edge id or -1,
             quad_node [nqtot] int16 local node idx)."""
    d = np.bincount(col_local_sorted, minlength=ns)
    q = -(-d // QUAD)
    nq = int(q.sum())
    assert nq <= nqtot, (nq, nqtot)
    qstart = np.zeros(ns + 1, np.int64)
    np.cumsum(q, out=qstart[1:])
    estart = np.zeros(ns + 1, np.int64)
    np.cumsum(d, out=estart[1:])
    within = np.arange(len(e_sorted)) - estart[col_local_sorted]
    spos = QUAD * qstart[col_local_sorted] + within
    eflat = np.full(nqtot * QUAD, -1, np.int64)
    eflat[spos] = e_sorted
    quad_node = np.zeros(nqtot, np.int16)
    quad_node[:nq] = np.repeat(
        np.arange(ns, dtype=np.int16), q.astype(np.int64))
    pk4 = nqtot // P
    eslot_dev = np.ascontiguousarray(
        eflat.reshape(pk4, P, QUAD).transpose(1, 0, 2)
    ).reshape(P, pk4 * QUAD)
    return eslot_dev, quad_node


def host_prep(inputs, cfg, dp, percore, acalls):
    edge_attr = np.asarray(inputs["edge_attr"], np.float32)
    x = np.asarray(inputs["x"], np.float32)
    W_lin = np.asarray(inputs["W_lin"], np.float32)
    b_lin = np.asarray(inputs["b_lin"], np.float32)
    W_edge = np.asarray(inputs["W_edge"], np.float32)
    b_edge = np.asarray(inputs["b_edge"], np.float32)

    n = cfg.n_nodes
    nt_all = cfg.ns * cfg.ncores
    xt_all = np.zeros((cfg.in_c + 1, nt_all), np.float32)
    xt_all[:cfg.in_c, :n] = x.T
    xt_all[cfg.in_c, :] = 1.0
    xt_all = _tobf16(xt_all)
    wlin_aug = _tobf16(np.concatenate([W_lin, b_lin[None, :]], 0))
    wedge_aug = np.concatenate([W_edge, b_edge[None, :]], 0)
    wedge_bd = np.zeros(((ED + 1) * KPACK, C * KPACK), np.float32)
    for j in range(KPACK):
        wedge_bd[j * (ED + 1):(j + 1) * (ED + 1), j * C:(j + 1) * C] = wedge_aug
    wedge_bd = _tobf16(wedge_bd)

    cv = np.zeros(64, np.float32)
    cv[:C] = dp["cw"]
    cv[16] = dp["b2"]
    if dp.get("quart_e") is not None:
        cv[17] = dp["quart_e"]["B"]
        cv[18] = dp["quart_e"]["gamma"]
        cv[19] = dp["quart_e"]["v"]
    elif dp["poly_e"] is not None:
        for i, (_a, b, _v) in enumerate(dp["poly_e"]):
            cv[17 + i] = b
    if dp.get("oct_h") is not None:
        cv[40] = dp["oct_h"]["B"]
        cv[41] = dp["oct_h"]["g1"]
        cv[42] = dp["oct_h"]["g2"]
        cv[43] = dp["oct_h"]["v"]
    elif dp["poly_h"] is not None:
        for i, (_a, b, _v) in enumerate(dp["poly_h"]):
            cv[40 + i] = b
    cvec_arr = np.broadcast_to(cv, (P, 64)).copy()

    in_maps = []
    for c in range(cfg.ncores):
        eslot_dev, quad_node = percore[c]
        # eat: [ED+1, pk, P] then K-pack
        m = eslot_dev >= 0                       # [P, pk]
        ea_pp = np.zeros((P, cfg.pk, ED), np.float32)
        ea_pp[m] = edge_attr[eslot_dev[m]]
        eat = np.empty((ED + 1, cfg.pk, P), np.float32)
        eat[:ED] = ea_pp.transpose(2, 1, 0)
        eat[ED] = 1.0
        eat8 = np.ascontiguousarray(
            eat.reshape(ED + 1, cfg.pk8, KPACK, P).transpose(2, 0, 1, 3)
        ).reshape((ED + 1) * KPACK, cfg.pk8, P)
        # quad idxs: call cc covers quads [cc*1024, (cc+1)*1024);
        # idx i at [i%16, cc*64 + i//16], replicated over 8 partition groups
        nch = cfg.nchunks
        nch0 = nch // 2
        qn = quad_node.astype(np.int32).reshape(cfg.ncalls, NI_CALL)
        mult = np.where(np.asarray(acalls)[:, None], nch0, nch)
        qrow = ((qn % P) * mult + qn // P).astype(np.int16)
        qi = qrow.reshape(cfg.ncalls, NI_CALL // 16, 16)
        qi = np.ascontiguousarray(qi.transpose(0, 2, 1)).reshape(
            cfg.ncalls * 16, NI_CALL // 16)
        qi = qi.reshape(cfg.ncalls, 16, NI_CALL // 16).transpose(
            1, 0, 2).reshape(16, cfg.icols)
        qidx_arr = np.tile(qi, (8, 1))
        in_maps.append({
            "xt": np.ascontiguousarray(xt_all[:, c * cfg.ns:(c + 1) * cfg.ns]),
            "wlin": wlin_aug,
            "eat": _tobf16(eat8),
            "wedge": wedge_bd,
            "qidx": qidx_arr,
            "cvec": cvec_arr,
        })
    return in_maps


def run(inputs, cfg=None, trace=False, kt=192):
    from concourse.bass_utils import run_bass_kernel_spmd
    from concourse import mybir

    _patch_compiler_flags()
    col = np.asarray(inputs["col"], np.int32)
    n_nodes = inputs["x"].shape[0]
    e_edges = col.shape[0]
    base = Cfg(n_nodes, e_edges, NCORES, kt=kt)
    ns = base.ns
    owner = np.minimum(col // ns, NCORES - 1)
    order = np.argsort(owner.astype(np.int64) * (n_nodes + 1) + col,
                       kind="stable")
    bounds = np.searchsorted(owner[order], np.arange(NCORES + 1))
    # per-core quad counts -> capacity
    caps = []
    sorted_cols = []
    for c in range(NCORES):
        eidx = order[bounds[c]:bounds[c + 1]]
        cl = col[eidx] - c * ns
        d = np.bincount(cl, minlength=ns)
        caps.append(int((-(-d // QUAD)).sum()))
        sorted_cols.append((cl, eidx))
    ecap = max(caps) * QUAD
    cfg = Cfg(n_nodes, e_edges, NCORES, kt=kt, ecap=ecap)

    percore = []
    for c in range(NCORES):
        cl, eidx = sorted_cols[c]
        percore.append(quad_layout(cl, eidx, ns, cfg.nqp * P))

    # a gather call may target the early half-table only if every core's
    # quads in that call live in the first nchunks//2 chunks
    nch0 = cfg.nchunks // 2
    acalls = []
    for k in range(cfg.ncalls):
        ok = True
        for c in range(NCORES):
            qn = percore[c][1][k * NI_CALL:(k + 1) * NI_CALL]
            if qn.size and int(qn.max()) // P >= nch0:
                ok = False
                break
        acalls.append(ok)

    dp = derive_params(inputs)
    nc = build_graph(cfg, dp, acalls)
    mybir.codegen_inst_isa_subclasses(nc)
    _split_multi_waits(nc)
    in_maps = host_prep(inputs, cfg, dp, percore, acalls)
    res = run_bass_kernel_spmd(nc, in_maps, list(range(cfg.ncores)),
                               trace=trace)
    full = np.empty((e_edges, 2 * C), np.float32)
    for c in range(cfg.ncores):
        eslot_dev, _ = percore[c]
        o = np.asarray(res.results[c]["out"], dtype=np.float32).reshape(
            P, cfg.pk, 2 * C)
        m = eslot_dev >= 0
        full[eslot_dev[m]] = o[m]
    return full, res


def kernel(**inputs):
    full, _ = run(inputs, trace=False)
    return full
